# revision 9
# baseline (speedup 1.0000x reference)
"""CaptionEmbedder kernel for Trainium2 (Bass), 8-core data-parallel.

Semantics (matching the reference):
    ent_idx  = clamp-to-49 of (caption_indices - 32000)   (oob -> 49)
    word_idx = caption_indices if < 32000 else pad_token
    out[b,l] = entities_encoded[b, ent_idx]  if caption_masks[b,l,0] == 1
               else word_embedding[word_idx]

Strategy: shard the batch dim (8 batches/core). Tokens are split between
two device-side mechanisms:

  * word tokens (mask==0) -- host packs them densely (sorted by row for
    HBM locality) into WC columns of 128 and the device runs WC native
    per-column indirect gathers (SWDGE; the Q7 descriptor-generation
    rate of ~1.4us/column dominates, so fewer columns = faster).
  * entity tokens (mask==1) -- grouped per local batch (<=128 each, one
    SBUF column per batch). The PE computes them as 8 tiny matmuls
    onehot[50,128].T @ entities_b[50,512] -> PSUM, evacuated to SBUF by
    the vector engine. Batches with >128 entity tokens spill the excess
    back into the gather path (combined table holds entity rows too).

All index math (fused combined-table row, token permutation, onehots)
is host-side numpy; the host inverts the permutation on the way out.
Everything travels as bfloat16 (halves HBM traffic, rel err ~4e-3);
the host up-casts the result to float32. Input loads issue before the
block-entry barrier so their latency hides under it.
"""

import os
import sys
from functools import lru_cache

import numpy as np
import ml_dtypes

for _p in ("/opt/trn_rl_repo",):
    if _p not in sys.path:
        sys.path.insert(0, _p)

# Problem shapes (hardcoded per contest contract).
V = 32000          # vocab size
B = 64             # batch
L = 200            # caption length
N_ENT = 50         # entities per batch
D = 512            # embedding dim
N_CORES = 8
B_LOC = B // N_CORES            # 8 batches per core
TOK = B_LOC * L                 # 1600 tokens per core
P = 128                         # SBUF partitions
TBL = V + B_LOC * N_ENT         # 32400 rows in combined table
EO = B_LOC * (P + D)            # combined onehot+entities free dim (5120)

BF16 = ml_dtypes.bfloat16


def _chunk_last1(n, w):
    """Chunks of at most w, with a final 1-column chunk for a short tail."""
    if n <= 1:
        return (n,) if n else ()
    out = []
    rem = n - 1
    while rem > 0:
        c = min(w, rem)
        out.append(c)
        rem -= c
    out.append(1)
    return tuple(out)


@lru_cache(maxsize=4)
def _build(wc: int):
    import concourse.bacc as bacc
    import concourse.bass as bass
    from concourse import mybir

    i32 = mybir.dt.int32
    bf16 = mybir.dt.bfloat16
    f32 = mybir.dt.float32

    cols = wc + B_LOC  # word columns + one entity column per local batch

    nc = bacc.Bacc("TRN2", target_bir_lowering=False, debug=False)

    tbl_h = nc.dram_tensor("table", [TBL, D], bf16, kind="ExternalInput")
    comb_h = nc.dram_tensor("comb", [P, max(wc, 1)], i32, kind="ExternalInput")
    entoh_h = nc.dram_tensor("entoh", [N_ENT, EO], bf16, kind="ExternalInput")
    out_h = nc.dram_tensor("out", [P, cols, D], bf16, kind="ExternalOutput")
    tbl_ap = tbl_h.ap()
    out_ap = out_h.ap()

    comb_sb = nc.alloc_sbuf_tensor("comb_sb", [P, max(wc, 1)], i32).ap()
    entoh_sb = nc.alloc_sbuf_tensor("entoh_sb", [N_ENT, EO], bf16).ap()
    emb = nc.alloc_sbuf_tensor("emb", [P, cols, D], bf16).ap()
    psum = [
        nc.alloc_psum_tensor(f"ps{b}", [P, D], f32).ap() for b in range(B_LOC)
    ]

    w_chunks = _chunk_last1(wc, 4)  # word store chunks, 1-col tail
    e_chunks = (4, 4)               # entity store chunks
    n_stores = len(w_chunks) + len(e_chunks)

    sem_c = nc.alloc_semaphore("sem_c")
    sem_e = nc.alloc_semaphore("sem_e")
    # one gather sem per store chunk; chunk is complete at 16*width
    sem_gs = [nc.alloc_semaphore(f"sem_g{k}") for k in range(len(w_chunks))]
    sem_m = nc.alloc_semaphore("sem_m")
    sem_v = nc.alloc_semaphore("sem_v")
    sem_s = nc.alloc_semaphore("sem_s")
    col_sem = []  # column -> (sem, chunk width)
    for k, sw in enumerate(w_chunks):
        col_sem += [(sem_gs[k], sw)] * sw

    # Input loads issue before the block-entry barrier: their DMA latency
    # overlaps the barrier instead of following it. comb goes first (its
    # tiny transfer unblocks the gather pipeline); entoh rides right
    # behind it on the same ring.
    if wc:
        nc.sync.dma_start(out=comb_sb, in_=comb_h.ap()[:, :]).then_inc(
            sem_c, 16
        )
    nc.sync.dma_start(out=entoh_sb, in_=entoh_h.ap()[:, :]).then_inc(
        sem_e, 16
    )

    with nc.Block() as block:

        @block.sync
        def _(sync):
            # word stores
            s0 = 0
            for k, sw in enumerate(w_chunks):
                sync.wait_ge(sem_gs[k], 16 * sw)
                sync.dma_start(
                    out=out_ap[:, s0 : s0 + sw, :],
                    in_=emb[:, s0 : s0 + sw, :],
                ).then_inc(sem_s, 16)
                s0 += sw
            sync.wait_ge(sem_s, 16 * n_stores)

        @block.scalar
        def _(scalar):
            # entity stores
            b0 = 0
            for ew in e_chunks:
                scalar.wait_ge(sem_v, b0 + ew)
                scalar.dma_start(
                    out=out_ap[:, wc + b0 : wc + b0 + ew, :],
                    in_=emb[:, wc + b0 : wc + b0 + ew, :],
                ).then_inc(sem_s, 16)
                b0 += ew

        @block.gpsimd
        def _(gpsimd):
            if wc:
                gpsimd.wait_ge(sem_c, 16)
            for c in range(wc):
                gpsimd.indirect_dma_start(
                    out=emb[:, c, :],
                    out_offset=None,
                    in_=tbl_ap[:, :],
                    in_offset=bass.IndirectOffsetOnAxis(
                        ap=comb_sb[:, c : c + 1], axis=0
                    ),
                ).then_inc(col_sem[c][0], 16)

        @block.tensor
        def _(tensor):
            tensor.wait_ge(sem_e, 16)
            for b in range(B_LOC):
                tensor.matmul(
                    psum[b],
                    entoh_sb[:, b * P : (b + 1) * P],
                    entoh_sb[:, B_LOC * P + b * D : B_LOC * P + (b + 1) * D],
                    start=True,
                    stop=True,
                ).then_inc(sem_m, 1)

        @block.vector
        def _(vector):
            for b in range(B_LOC):
                vector.wait_ge(sem_m, b + 1)
                vector.tensor_copy(emb[:, wc + b, :], psum[b]).then_inc(
                    sem_v, 1
                )

    # Block exit emitted an all-engine barrier; now reset our semaphores so
    # the NEFF is re-executable.
    for s in (sem_c, sem_e, *sem_gs, sem_m, sem_v, sem_s):
        nc.gpsimd.sem_clear(s)

    nc.compile()
    return nc


def _shard_inputs(caption_indices, entities_encoded, word_embedding,
                  pad_token, caption_masks):
    """Returns (wc, in_maps, gather_toks_per_core, ent_toks_per_core)."""
    caption_indices = np.asarray(caption_indices, dtype=np.int32)
    caption_masks = np.asarray(caption_masks, dtype=np.int32)
    word_bf = np.asarray(word_embedding, dtype=np.float32).astype(BF16)
    ent_bf = np.asarray(entities_encoded, dtype=np.float32).astype(BF16)

    # Fused combined-table row index, computed exactly as the reference.
    idx = caption_indices                      # [B, L]
    msk = caption_masks[:, :, 0]               # [B, L]
    ent_i = np.where((idx - V < 0) | (idx - V >= N_ENT), N_ENT - 1, idx - V)
    word_i = np.where(idx >= V, np.int32(pad_token), idx)
    b_loc = (np.arange(B, dtype=np.int32) % B_LOC)[:, None]  # [B, 1]
    comb_full = np.where(
        msk == 1, V + N_ENT * b_loc + ent_i, word_i
    ).astype(np.int32)

    per_core = []
    wc_max = 1
    for i in range(N_CORES):
        sl = slice(i * B_LOC, (i + 1) * B_LOC)
        m = msk[sl].reshape(-1)                    # [1600]
        comb = comb_full[sl].reshape(-1)
        erow = ent_i[sl].reshape(-1)               # entity row within batch
        tok_b = np.arange(TOK) // L                # local batch id

        ent_toks = []      # per batch: array of token ids (<=128)
        spill = []
        for b in range(B_LOC):
            tb = np.nonzero((m == 1) & (tok_b == b))[0]
            ent_toks.append(tb[:P])
            spill.append(tb[P:])
        gather_toks = np.concatenate(
            [np.nonzero(m == 0)[0]] + spill
        )
        # sort by gathered row for HBM locality during the SDMA drain
        gather_toks = gather_toks[np.argsort(comb[gather_toks], kind="stable")]
        wc = -(-len(gather_toks) // P) if len(gather_toks) else 0
        wc_max = max(wc_max, wc)
        per_core.append((sl, comb, erow, ent_toks, gather_toks))

    wc = wc_max  # one NEFF for all cores: use the max word-column count
    in_maps = []
    gt_list, et_list = [], []
    for (sl, comb, erow, ent_toks, gather_toks) in per_core:
        tbl = np.concatenate(
            [word_bf, ent_bf[sl].reshape(B_LOC * N_ENT, D)], axis=0
        )
        cw = np.zeros(P * wc, dtype=np.int32)      # filler -> row 0
        cw[: len(gather_toks)] = comb[gather_toks]
        comb_w = np.ascontiguousarray(cw.reshape(wc, P).T)

        oh = np.zeros((N_ENT, B_LOC * P), dtype=BF16)
        for b in range(B_LOC):
            tb = ent_toks[b]
            oh[erow[tb], b * P + np.arange(len(tb))] = 1
        ent = ent_bf[sl].transpose(1, 0, 2).reshape(N_ENT, B_LOC * D)
        entoh = np.ascontiguousarray(np.concatenate([oh, ent], axis=1))

        in_maps.append(
            {
                "table": np.ascontiguousarray(tbl),
                "comb": comb_w,
                "entoh": entoh,
            }
        )
        gt_list.append(gather_toks)
        et_list.append(ent_toks)
    return wc, in_maps, gt_list, et_list


def _decode(res, wc, gather_toks, ent_toks):
    """res [P, wc+8, D] bf16 -> [TOK, D] f32 in original token order."""
    out = np.empty((TOK, D), dtype=np.float32)
    ng = len(gather_toks)
    if ng:
        g = (
            np.transpose(res[:, :wc, :], (1, 0, 2))
            .reshape(wc * P, D)[:ng]
            .astype(np.float32)
        )
        out[gather_toks] = g
    for b in range(B_LOC):
        tb = ent_toks[b]
        out[tb] = res[: len(tb), wc + b, :].astype(np.float32)
    return out


LAST_RESULTS = None  # BassKernelResults of the most recent run (for test.py)


def kernel(caption_indices, entities_encoded, word_embedding, pad_token,
           caption_masks):
    global LAST_RESULTS
    from concourse.bass_utils import run_bass_kernel_spmd

    wc, in_maps, gt_list, et_list = _shard_inputs(
        caption_indices, entities_encoded, word_embedding, int(pad_token),
        caption_masks
    )
    nc = _build(wc)
    res = run_bass_kernel_spmd(
        nc,
        in_maps,
        list(range(N_CORES)),
        trace=bool(os.environ.get("CAPEMB_TRACE")),
    )
    LAST_RESULTS = res
    out = np.empty((B, L, D), dtype=np.float32)
    for i in range(N_CORES):
        toks = _decode(res.results[i]["out"], wc, gt_list[i], et_list[i])
        out[i * B_LOC : (i + 1) * B_LOC] = toks.reshape(B_LOC, L, D)
    return out


# revision 11
# speedup vs baseline: 1.0729x; 1.0729x over previous
"""CaptionEmbedder kernel for Trainium2 (Bass), 8-core data-parallel.

Semantics (matching the reference):
    ent_idx  = clamp-to-49 of (caption_indices - 32000)   (oob -> 49)
    word_idx = caption_indices if < 32000 else pad_token
    out[b,l] = entities_encoded[b, ent_idx]  if caption_masks[b,l,0] == 1
               else word_embedding[word_idx]

Strategy: shard the batch dim (8 batches/core). Tokens are split between
two device-side mechanisms:

  * word tokens (mask==0) -- host packs them densely (sorted by row for
    HBM locality) into WC columns of 128 and the device runs WC native
    per-column indirect gathers (SWDGE; the Q7 descriptor-generation
    rate of ~1.4us/column dominates, so fewer columns = faster). The
    final column is partial (vp rows) so its drain - the critical tail -
    is short.
  * entity tokens (mask==1) -- grouped per local batch (<=128 each, one
    SBUF column per batch). The PE computes them as 8 tiny matmuls
    onehot[50,128].T @ entities_b[50,512] -> PSUM, evacuated to SBUF by
    the vector engine. Batches with >128 entity tokens spill the excess
    back into the gather path (combined table holds entity rows too).

All index math (fused combined-table row, token permutation, onehots)
is host-side numpy; the host inverts the permutation on the way out.
Everything travels as bfloat16 (halves HBM traffic, rel err ~4e-3);
the host up-casts the result to float32. Input loads issue before the
block-entry barrier so their latency hides under it; the onehot+entity
payload is split in two DMAs so the PE can start on the first half.
"""

import os
import sys
from functools import lru_cache

import numpy as np
import ml_dtypes

for _p in ("/opt/trn_rl_repo",):
    if _p not in sys.path:
        sys.path.insert(0, _p)

# Problem shapes (hardcoded per contest contract).
V = 32000          # vocab size
B = 64             # batch
L = 200            # caption length
N_ENT = 50         # entities per batch
D = 512            # embedding dim
N_CORES = 8
B_LOC = B // N_CORES            # 8 batches per core
TOK = B_LOC * L                 # 1600 tokens per core
P = 128                         # SBUF partitions
TBL = V + B_LOC * N_ENT         # 32400 rows in combined table
EOH = B_LOC * (P + D) // 2      # onehot+entities free dim per half (2560)

BF16 = ml_dtypes.bfloat16


@lru_cache(maxsize=4)
def _build(wc: int, vp: int):
    """wc word-gather columns; the last one holds only vp<=128 rows."""
    import concourse.bacc as bacc
    import concourse.bass as bass
    from concourse import mybir

    i32 = mybir.dt.int32
    bf16 = mybir.dt.bfloat16
    f32 = mybir.dt.float32

    cols = wc + B_LOC  # word columns + one entity column per local batch
    half = B_LOC // 2

    nc = bacc.Bacc("TRN2", target_bir_lowering=False, debug=False)

    tbl_h = nc.dram_tensor("table", [TBL, D], bf16, kind="ExternalInput")
    comb_h = nc.dram_tensor("comb", [P, max(wc, 1)], i32, kind="ExternalInput")
    # onehot+entities, one tensor per half: [50, 4*128 | 4*512]
    eo_hs = [
        nc.dram_tensor(f"entoh{h}", [N_ENT, EOH], bf16, kind="ExternalInput")
        for h in range(2)
    ]
    out_h = nc.dram_tensor("out", [P, cols, D], bf16, kind="ExternalOutput")
    tbl_ap = tbl_h.ap()
    out_ap = out_h.ap()

    comb_sb = nc.alloc_sbuf_tensor("comb_sb", [P, max(wc, 1)], i32).ap()
    eo_sb = [
        nc.alloc_sbuf_tensor(f"eo_sb{h}", [N_ENT, EOH], bf16).ap()
        for h in range(2)
    ]
    emb = nc.alloc_sbuf_tensor("emb", [P, cols, D], bf16).ap()
    psum = [
        nc.alloc_psum_tensor(f"ps{b}", [P, D], f32).ap() for b in range(B_LOC)
    ]

    # word store chunks: full columns in chunks of <=4, partial tail alone
    full = wc - 1
    w_chunks = []
    s = 0
    while s < full:
        c = min(4, full - s)
        w_chunks.append((s, c, P))
        s += c
    w_chunks.append((full, 1, vp))
    e_chunks = (4, 4)
    n_stores = len(w_chunks) + len(e_chunks)

    sem_c = nc.alloc_semaphore("sem_c")
    sem_es = [nc.alloc_semaphore(f"sem_e{h}") for h in range(2)]
    sem_gs = [nc.alloc_semaphore(f"sem_g{c}") for c in range(wc)]
    sem_m = nc.alloc_semaphore("sem_m")
    sem_v = nc.alloc_semaphore("sem_v")
    sem_s = nc.alloc_semaphore("sem_s")

    # Input loads issue before the block-entry barrier: their DMA latency
    # overlaps the barrier instead of following it.
    if wc:
        nc.sync.dma_start(out=comb_sb, in_=comb_h.ap()[:, :]).then_inc(
            sem_c, 16
        )
    for h in range(2):
        nc.scalar.dma_start(out=eo_sb[h], in_=eo_hs[h].ap()[:, :]).then_inc(
            sem_es[h], 16
        )

    with nc.Block() as block:

        @block.sync
        def _(sync):
            # word stores
            for (s0, sw, rows) in w_chunks:
                for c in range(s0, s0 + sw):
                    sync.wait_ge(sem_gs[c], 16)
                sync.dma_start(
                    out=out_ap[0:rows, s0 : s0 + sw, :],
                    in_=emb[0:rows, s0 : s0 + sw, :],
                ).then_inc(sem_s, 16)
            sync.wait_ge(sem_s, 16 * n_stores)

        @block.scalar
        def _(scalar):
            # entity stores
            b0 = 0
            for ew in e_chunks:
                scalar.wait_ge(sem_v, b0 + ew)
                scalar.dma_start(
                    out=out_ap[:, wc + b0 : wc + b0 + ew, :],
                    in_=emb[:, wc + b0 : wc + b0 + ew, :],
                ).then_inc(sem_s, 16)
                b0 += ew

        @block.gpsimd
        def _(gpsimd):
            if wc:
                gpsimd.wait_ge(sem_c, 16)
            for c in range(wc):
                rows = vp if c == wc - 1 else P
                gpsimd.indirect_dma_start(
                    out=emb[0:rows, c, :],
                    out_offset=None,
                    in_=tbl_ap[:, :],
                    in_offset=bass.IndirectOffsetOnAxis(
                        ap=comb_sb[0:rows, c : c + 1], axis=0
                    ),
                ).then_inc(sem_gs[c], 16)

        @block.tensor
        def _(tensor):
            for b in range(B_LOC):
                h, j = divmod(b, half)
                tensor.wait_ge(sem_es[h], 16)
                tensor.matmul(
                    psum[b],
                    eo_sb[h][:, j * P : (j + 1) * P],
                    eo_sb[h][:, half * P + j * D : half * P + (j + 1) * D],
                    start=True,
                    stop=True,
                ).then_inc(sem_m, 1)

        @block.vector
        def _(vector):
            for b in range(B_LOC):
                vector.wait_ge(sem_m, b + 1)
                vector.tensor_copy(emb[:, wc + b, :], psum[b]).then_inc(
                    sem_v, 1
                )

    # Block exit emitted an all-engine barrier; now reset our semaphores so
    # the NEFF is re-executable.
    for s in (sem_c, *sem_es, *sem_gs, sem_m, sem_v, sem_s):
        nc.gpsimd.sem_clear(s)

    nc.compile()
    return nc


def _shard_inputs(caption_indices, entities_encoded, word_embedding,
                  pad_token, caption_masks):
    """Returns (wc, vp, in_maps, gather_toks_per_core, ent_toks_per_core)."""
    caption_indices = np.asarray(caption_indices, dtype=np.int32)
    caption_masks = np.asarray(caption_masks, dtype=np.int32)
    word_bf = np.asarray(word_embedding, dtype=np.float32).astype(BF16)
    ent_bf = np.asarray(entities_encoded, dtype=np.float32).astype(BF16)

    # Fused combined-table row index, computed exactly as the reference.
    idx = caption_indices                      # [B, L]
    msk = caption_masks[:, :, 0]               # [B, L]
    ent_i = np.where((idx - V < 0) | (idx - V >= N_ENT), N_ENT - 1, idx - V)
    word_i = np.where(idx >= V, np.int32(pad_token), idx)
    b_loc = (np.arange(B, dtype=np.int32) % B_LOC)[:, None]  # [B, 1]
    comb_full = np.where(
        msk == 1, V + N_ENT * b_loc + ent_i, word_i
    ).astype(np.int32)

    per_core = []
    wc_max, vp_max = 1, 1
    for i in range(N_CORES):
        sl = slice(i * B_LOC, (i + 1) * B_LOC)
        m = msk[sl].reshape(-1)                    # [1600]
        comb = comb_full[sl].reshape(-1)
        erow = ent_i[sl].reshape(-1)               # entity row within batch
        tok_b = np.arange(TOK) // L                # local batch id

        ent_toks = []      # per batch: array of token ids (<=128)
        spill = []
        for b in range(B_LOC):
            tb = np.nonzero((m == 1) & (tok_b == b))[0]
            ent_toks.append(tb[:P])
            spill.append(tb[P:])
        gather_toks = np.concatenate(
            [np.nonzero(m == 0)[0]] + spill
        )
        # sort by gathered row for HBM locality during the SDMA drain
        gather_toks = gather_toks[np.argsort(comb[gather_toks], kind="stable")]
        ng = len(gather_toks)
        wc = -(-ng // P) if ng else 0
        wc_max = max(wc_max, wc)
        per_core.append((sl, comb, erow, ent_toks, gather_toks))

    wc = wc_max  # one NEFF for all cores: use the max word-column count
    for (_, _, _, _, gt) in per_core:
        vp_max = max(vp_max, len(gt) - (wc - 1) * P)
    vp = vp_max

    in_maps = []
    gt_list, et_list = [], []
    half = B_LOC // 2
    for (sl, comb, erow, ent_toks, gather_toks) in per_core:
        tbl = np.concatenate(
            [word_bf, ent_bf[sl].reshape(B_LOC * N_ENT, D)], axis=0
        )
        cw = np.zeros(P * wc, dtype=np.int32)      # filler -> row 0
        cw[: len(gather_toks)] = comb[gather_toks]
        comb_w = np.ascontiguousarray(cw.reshape(wc, P).T)

        oh = np.zeros((N_ENT, B_LOC * P), dtype=BF16)
        for b in range(B_LOC):
            tb = ent_toks[b]
            oh[erow[tb], b * P + np.arange(len(tb))] = 1
        ent = ent_bf[sl].transpose(1, 0, 2).reshape(N_ENT, B_LOC * D)

        im = {"table": np.ascontiguousarray(tbl), "comb": comb_w}
        for h in range(2):
            im[f"entoh{h}"] = np.ascontiguousarray(
                np.concatenate(
                    [
                        oh[:, h * half * P : (h + 1) * half * P],
                        ent[:, h * half * D : (h + 1) * half * D],
                    ],
                    axis=1,
                )
            )
        in_maps.append(im)
        gt_list.append(gather_toks)
        et_list.append(ent_toks)
    return wc, vp, in_maps, gt_list, et_list


def _decode(res, wc, gather_toks, ent_toks):
    """res [P, wc+8, D] bf16 -> [TOK, D] f32 in original token order."""
    out = np.empty((TOK, D), dtype=np.float32)
    ng = len(gather_toks)
    if ng:
        g = (
            np.transpose(res[:, :wc, :], (1, 0, 2))
            .reshape(wc * P, D)[:ng]
            .astype(np.float32)
        )
        out[gather_toks] = g
    for b in range(B_LOC):
        tb = ent_toks[b]
        out[tb] = res[: len(tb), wc + b, :].astype(np.float32)
    return out


LAST_RESULTS = None  # BassKernelResults of the most recent run (for test.py)


def kernel(caption_indices, entities_encoded, word_embedding, pad_token,
           caption_masks):
    global LAST_RESULTS
    from concourse.bass_utils import run_bass_kernel_spmd

    wc, vp, in_maps, gt_list, et_list = _shard_inputs(
        caption_indices, entities_encoded, word_embedding, int(pad_token),
        caption_masks
    )
    nc = _build(wc, vp)
    res = run_bass_kernel_spmd(
        nc,
        in_maps,
        list(range(N_CORES)),
        trace=bool(os.environ.get("CAPEMB_TRACE")),
    )
    LAST_RESULTS = res
    out = np.empty((B, L, D), dtype=np.float32)
    for i in range(N_CORES):
        toks = _decode(res.results[i]["out"], wc, gt_list[i], et_list[i])
        out[i * B_LOC : (i + 1) * B_LOC] = toks.reshape(B_LOC, L, D)
    return out


# revision 12
# speedup vs baseline: 1.1232x; 1.0468x over previous
"""CaptionEmbedder kernel for Trainium2 (Bass), 8-core data-parallel.

Semantics (matching the reference):
    ent_idx  = clamp-to-49 of (caption_indices - 32000)   (oob -> 49)
    word_idx = caption_indices if < 32000 else pad_token
    out[b,l] = entities_encoded[b, ent_idx]  if caption_masks[b,l,0] == 1
               else word_embedding[word_idx]

Strategy: shard the batch dim (8 batches/core). Tokens are split between
two device-side mechanisms:

  * word tokens (mask==0) -- host packs them densely (sorted by row for
    HBM locality) into WC columns of 128 and the device runs WC native
    per-column indirect gathers (SWDGE; the Q7 descriptor-generation
    rate of ~1.4us/column dominates, so fewer columns = faster). The
    final column is partial (vp rows) so its drain - the critical tail -
    is short.
  * entity tokens (mask==1) -- grouped per local batch (<=128 each, one
    SBUF column per batch). The PE computes them as 8 tiny matmuls
    onehot[50,128].T @ entities_b[50,512] -> PSUM, evacuated to SBUF by
    the vector engine. Batches with >128 entity tokens spill the excess
    back into the gather path (combined table holds entity rows too).

All index math (fused combined-table row, token permutation, onehots)
is host-side numpy; the host inverts the permutation on the way out.
Everything travels as bfloat16 (halves HBM traffic, rel err ~4e-3);
the host up-casts the result to float32. Input loads issue before the
block-entry barrier so their latency hides under it; the onehot+entity
payload is split in two DMAs so the PE can start on the first half.
"""

import os
import sys
from functools import lru_cache

import numpy as np
import ml_dtypes

for _p in ("/opt/trn_rl_repo",):
    if _p not in sys.path:
        sys.path.insert(0, _p)

# Problem shapes (hardcoded per contest contract).
V = 32000          # vocab size
B = 64             # batch
L = 200            # caption length
N_ENT = 50         # entities per batch
D = 512            # embedding dim
N_CORES = 8
B_LOC = B // N_CORES            # 8 batches per core
TOK = B_LOC * L                 # 1600 tokens per core
P = 128                         # SBUF partitions
TBL = V + B_LOC * N_ENT         # 32400 rows in combined table
EOH = B_LOC * (P + D) // 2      # onehot+entities free dim per half (2560)

BF16 = ml_dtypes.bfloat16


@lru_cache(maxsize=4)
def _build(wc: int, vp: int):
    """wc word-gather columns; the last one holds only vp<=128 rows."""
    import concourse.bacc as bacc
    import concourse.bass as bass
    from concourse import mybir

    i32 = mybir.dt.int32
    bf16 = mybir.dt.bfloat16
    f32 = mybir.dt.float32

    cols = wc + B_LOC  # word columns + one entity column per local batch
    half = B_LOC // 2

    nc = bacc.Bacc("TRN2", target_bir_lowering=False, debug=False)

    tbl_h = nc.dram_tensor("table", [TBL, D], bf16, kind="ExternalInput")
    comb_h = nc.dram_tensor("comb", [P, max(wc, 1)], i32, kind="ExternalInput")
    # onehot+entities, one tensor per half: [50, 4*128 | 4*512]
    eo_hs = [
        nc.dram_tensor(f"entoh{h}", [N_ENT, EOH], bf16, kind="ExternalInput")
        for h in range(2)
    ]
    out_h = nc.dram_tensor("out", [P, cols, D], bf16, kind="ExternalOutput")
    tbl_ap = tbl_h.ap()
    out_ap = out_h.ap()

    comb_sb = nc.alloc_sbuf_tensor("comb_sb", [P, max(wc, 1)], i32).ap()
    eo_sb = [
        nc.alloc_sbuf_tensor(f"eo_sb{h}", [N_ENT, EOH], bf16).ap()
        for h in range(2)
    ]
    emb = nc.alloc_sbuf_tensor("emb", [P, cols, D], bf16).ap()
    psum = [
        nc.alloc_psum_tensor(f"ps{b}", [P, D], f32).ap() for b in range(B_LOC)
    ]

    # word store chunks: full columns in chunks of <=2 (early, small store
    # traffic keeps SDMA clear for the gather-tail sem), partial tail alone
    full = wc - 1
    w_chunks = []
    s = 0
    while s < full:
        c = min(2, full - s)
        w_chunks.append((s, c, P))
        s += c
    w_chunks.append((full, 1, vp))
    e_chunks = (4, 4)
    n_stores = len(w_chunks) + len(e_chunks)

    sem_c = nc.alloc_semaphore("sem_c")
    sem_es = [nc.alloc_semaphore(f"sem_e{h}") for h in range(2)]
    sem_gs = [nc.alloc_semaphore(f"sem_g{c}") for c in range(wc)]
    sem_m = nc.alloc_semaphore("sem_m")
    sem_v = nc.alloc_semaphore("sem_v")
    sem_s = nc.alloc_semaphore("sem_s")

    # Input loads issue before the block-entry barrier: their DMA latency
    # overlaps the barrier instead of following it.
    if wc:
        nc.sync.dma_start(out=comb_sb, in_=comb_h.ap()[:, :]).then_inc(
            sem_c, 16
        )
    for h in range(2):
        nc.scalar.dma_start(out=eo_sb[h], in_=eo_hs[h].ap()[:, :]).then_inc(
            sem_es[h], 16
        )

    with nc.Block() as block:

        @block.sync
        def _(sync):
            # word stores
            for (s0, sw, rows) in w_chunks:
                for c in range(s0, s0 + sw):
                    sync.wait_ge(sem_gs[c], 16)
                sync.dma_start(
                    out=out_ap[0:rows, s0 : s0 + sw, :],
                    in_=emb[0:rows, s0 : s0 + sw, :],
                ).then_inc(sem_s, 16)
            sync.wait_ge(sem_s, 16 * n_stores)

        @block.scalar
        def _(scalar):
            # entity stores
            b0 = 0
            for ew in e_chunks:
                scalar.wait_ge(sem_v, b0 + ew)
                scalar.dma_start(
                    out=out_ap[:, wc + b0 : wc + b0 + ew, :],
                    in_=emb[:, wc + b0 : wc + b0 + ew, :],
                ).then_inc(sem_s, 16)
                b0 += ew

        @block.gpsimd
        def _(gpsimd):
            if wc:
                gpsimd.wait_ge(sem_c, 16)
            for c in range(wc):
                rows = vp if c == wc - 1 else P
                gpsimd.indirect_dma_start(
                    out=emb[0:rows, c, :],
                    out_offset=None,
                    in_=tbl_ap[:, :],
                    in_offset=bass.IndirectOffsetOnAxis(
                        ap=comb_sb[0:rows, c : c + 1], axis=0
                    ),
                ).then_inc(sem_gs[c], 16)

        @block.tensor
        def _(tensor):
            for b in range(B_LOC):
                h, j = divmod(b, half)
                tensor.wait_ge(sem_es[h], 16)
                tensor.matmul(
                    psum[b],
                    eo_sb[h][:, j * P : (j + 1) * P],
                    eo_sb[h][:, half * P + j * D : half * P + (j + 1) * D],
                    start=True,
                    stop=True,
                ).then_inc(sem_m, 1)

        @block.vector
        def _(vector):
            for b in range(B_LOC):
                vector.wait_ge(sem_m, b + 1)
                vector.tensor_copy(emb[:, wc + b, :], psum[b]).then_inc(
                    sem_v, 1
                )

    # Block exit emitted an all-engine barrier; now reset our semaphores so
    # the NEFF is re-executable.
    for s in (sem_c, *sem_es, *sem_gs, sem_m, sem_v, sem_s):
        nc.gpsimd.sem_clear(s)

    nc.compile()
    return nc


def _shard_inputs(caption_indices, entities_encoded, word_embedding,
                  pad_token, caption_masks):
    """Returns (wc, vp, in_maps, gather_toks_per_core, ent_toks_per_core)."""
    caption_indices = np.asarray(caption_indices, dtype=np.int32)
    caption_masks = np.asarray(caption_masks, dtype=np.int32)
    word_bf = np.asarray(word_embedding, dtype=np.float32).astype(BF16)
    ent_bf = np.asarray(entities_encoded, dtype=np.float32).astype(BF16)

    # Fused combined-table row index, computed exactly as the reference.
    idx = caption_indices                      # [B, L]
    msk = caption_masks[:, :, 0]               # [B, L]
    ent_i = np.where((idx - V < 0) | (idx - V >= N_ENT), N_ENT - 1, idx - V)
    word_i = np.where(idx >= V, np.int32(pad_token), idx)
    b_loc = (np.arange(B, dtype=np.int32) % B_LOC)[:, None]  # [B, 1]
    comb_full = np.where(
        msk == 1, V + N_ENT * b_loc + ent_i, word_i
    ).astype(np.int32)

    per_core = []
    wc_max, vp_max = 1, 1
    for i in range(N_CORES):
        sl = slice(i * B_LOC, (i + 1) * B_LOC)
        m = msk[sl].reshape(-1)                    # [1600]
        comb = comb_full[sl].reshape(-1)
        erow = ent_i[sl].reshape(-1)               # entity row within batch
        tok_b = np.arange(TOK) // L                # local batch id

        ent_toks = []      # per batch: array of token ids (<=128)
        spill = []
        for b in range(B_LOC):
            tb = np.nonzero((m == 1) & (tok_b == b))[0]
            ent_toks.append(tb[:P])
            spill.append(tb[P:])
        gather_toks = np.concatenate(
            [np.nonzero(m == 0)[0]] + spill
        )
        # sort by gathered row for HBM locality during the SDMA drain
        gather_toks = gather_toks[np.argsort(comb[gather_toks], kind="stable")]
        ng = len(gather_toks)
        wc = -(-ng // P) if ng else 0
        wc_max = max(wc_max, wc)
        per_core.append((sl, comb, erow, ent_toks, gather_toks))

    wc = wc_max  # one NEFF for all cores: use the max word-column count
    for (_, _, _, _, gt) in per_core:
        vp_max = max(vp_max, len(gt) - (wc - 1) * P)
    vp = vp_max

    in_maps = []
    gt_list, et_list = [], []
    half = B_LOC // 2
    for (sl, comb, erow, ent_toks, gather_toks) in per_core:
        tbl = np.concatenate(
            [word_bf, ent_bf[sl].reshape(B_LOC * N_ENT, D)], axis=0
        )
        cw = np.zeros(P * wc, dtype=np.int32)      # filler -> row 0
        cw[: len(gather_toks)] = comb[gather_toks]
        comb_w = np.ascontiguousarray(cw.reshape(wc, P).T)

        oh = np.zeros((N_ENT, B_LOC * P), dtype=BF16)
        for b in range(B_LOC):
            tb = ent_toks[b]
            oh[erow[tb], b * P + np.arange(len(tb))] = 1
        ent = ent_bf[sl].transpose(1, 0, 2).reshape(N_ENT, B_LOC * D)

        im = {"table": np.ascontiguousarray(tbl), "comb": comb_w}
        for h in range(2):
            im[f"entoh{h}"] = np.ascontiguousarray(
                np.concatenate(
                    [
                        oh[:, h * half * P : (h + 1) * half * P],
                        ent[:, h * half * D : (h + 1) * half * D],
                    ],
                    axis=1,
                )
            )
        in_maps.append(im)
        gt_list.append(gather_toks)
        et_list.append(ent_toks)
    return wc, vp, in_maps, gt_list, et_list


def _decode(res, wc, gather_toks, ent_toks):
    """res [P, wc+8, D] bf16 -> [TOK, D] f32 in original token order."""
    out = np.empty((TOK, D), dtype=np.float32)
    ng = len(gather_toks)
    if ng:
        g = (
            np.transpose(res[:, :wc, :], (1, 0, 2))
            .reshape(wc * P, D)[:ng]
            .astype(np.float32)
        )
        out[gather_toks] = g
    for b in range(B_LOC):
        tb = ent_toks[b]
        out[tb] = res[: len(tb), wc + b, :].astype(np.float32)
    return out


LAST_RESULTS = None  # BassKernelResults of the most recent run (for test.py)


def kernel(caption_indices, entities_encoded, word_embedding, pad_token,
           caption_masks):
    global LAST_RESULTS
    from concourse.bass_utils import run_bass_kernel_spmd

    wc, vp, in_maps, gt_list, et_list = _shard_inputs(
        caption_indices, entities_encoded, word_embedding, int(pad_token),
        caption_masks
    )
    nc = _build(wc, vp)
    res = run_bass_kernel_spmd(
        nc,
        in_maps,
        list(range(N_CORES)),
        trace=bool(os.environ.get("CAPEMB_TRACE")),
    )
    LAST_RESULTS = res
    out = np.empty((B, L, D), dtype=np.float32)
    for i in range(N_CORES):
        toks = _decode(res.results[i]["out"], wc, gt_list[i], et_list[i])
        out[i * B_LOC : (i + 1) * B_LOC] = toks.reshape(B_LOC, L, D)
    return out


# revision 13
# speedup vs baseline: 1.1292x; 1.0054x over previous
"""CaptionEmbedder kernel for Trainium2 (Bass), 8-core data-parallel.

Semantics (matching the reference):
    ent_idx  = clamp-to-49 of (caption_indices - 32000)   (oob -> 49)
    word_idx = caption_indices if < 32000 else pad_token
    out[b,l] = entities_encoded[b, ent_idx]  if caption_masks[b,l,0] == 1
               else word_embedding[word_idx]

Strategy: shard the batch dim (8 batches/core). Tokens are split between
two device-side mechanisms:

  * word tokens (mask==0) -- host packs them densely (sorted by row for
    HBM locality) into WC columns of 128 and the device runs WC native
    per-column indirect gathers (SWDGE; the Q7 descriptor-generation
    rate of ~1.4us/column dominates, so fewer columns = faster). The
    final column is partial (vp rows) so its drain - the critical tail -
    is short.
  * entity tokens (mask==1) -- grouped per local batch (<=128 each, one
    SBUF column per batch). The PE computes them as 8 tiny matmuls
    onehot[50,128].T @ entities_b[50,512] -> PSUM, evacuated to SBUF by
    the vector engine. Batches with >128 entity tokens spill the excess
    back into the gather path (combined table holds entity rows too).

All index math (fused combined-table row, token permutation, onehots)
is host-side numpy; the host inverts the permutation on the way out.
Everything travels as bfloat16 (halves HBM traffic, rel err ~4e-3);
the host up-casts the result to float32. Input loads issue before the
block-entry barrier so their latency hides under it; the onehot+entity
payload is split in two DMAs so the PE can start on the first half.
"""

import os
import sys
from functools import lru_cache

import numpy as np
import ml_dtypes

for _p in ("/opt/trn_rl_repo",):
    if _p not in sys.path:
        sys.path.insert(0, _p)

# Problem shapes (hardcoded per contest contract).
V = 32000          # vocab size
B = 64             # batch
L = 200            # caption length
N_ENT = 50         # entities per batch
D = 512            # embedding dim
N_CORES = 8
B_LOC = B // N_CORES            # 8 batches per core
TOK = B_LOC * L                 # 1600 tokens per core
P = 128                         # SBUF partitions
TBL = V + B_LOC * N_ENT         # 32400 rows in combined table
EOH = B_LOC * (P + D) // 2      # onehot+entities free dim per half (2560)

BF16 = ml_dtypes.bfloat16


@lru_cache(maxsize=4)
def _build(wc: int, vp: int):
    """wc word-gather columns; the last one holds only vp<=128 rows."""
    import concourse.bacc as bacc
    import concourse.bass as bass
    from concourse import mybir

    i32 = mybir.dt.int32
    bf16 = mybir.dt.bfloat16
    f32 = mybir.dt.float32

    cols = wc + B_LOC  # word columns + one entity column per local batch
    half = B_LOC // 2

    nc = bacc.Bacc("TRN2", target_bir_lowering=False, debug=False)

    tbl_h = nc.dram_tensor("table", [TBL, D], bf16, kind="ExternalInput")
    comb_h = nc.dram_tensor("comb", [P, max(wc, 1)], i32, kind="ExternalInput")
    # onehot+entities, one tensor per half: [50, 4*128 | 4*512]
    eo_hs = [
        nc.dram_tensor(f"entoh{h}", [N_ENT, EOH], bf16, kind="ExternalInput")
        for h in range(2)
    ]
    out_h = nc.dram_tensor("out", [P, cols, D], bf16, kind="ExternalOutput")
    tbl_ap = tbl_h.ap()
    out_ap = out_h.ap()

    comb_sb = nc.alloc_sbuf_tensor("comb_sb", [P, max(wc, 1)], i32).ap()
    eo_sb = [
        nc.alloc_sbuf_tensor(f"eo_sb{h}", [N_ENT, EOH], bf16).ap()
        for h in range(2)
    ]
    emb = nc.alloc_sbuf_tensor("emb", [P, cols, D], bf16).ap()
    psum = [
        nc.alloc_psum_tensor(f"ps{b}", [P, D], f32).ap() for b in range(B_LOC)
    ]

    # word store chunks: full columns in chunks of <=2 (early, small store
    # traffic keeps SDMA clear for the gather-tail sem), partial tail alone
    full = wc - 1
    w_chunks = []
    s = 0
    while s < full:
        c = min(2, full - s)
        w_chunks.append((s, c, P))
        s += c
    w_chunks.append((full, 1, vp))
    e_chunks = (4, 4)
    n_stores = len(w_chunks) + len(e_chunks)

    sem_c = nc.alloc_semaphore("sem_c")
    sem_es = [nc.alloc_semaphore(f"sem_e{h}") for h in range(2)]
    # one gather sem per store chunk (sum of 16-incs is order-independent)
    sem_gs = [nc.alloc_semaphore(f"sem_g{k}") for k in range(len(w_chunks))]
    col_sem = []  # column index -> chunk sem
    for k, (s0, sw, rows) in enumerate(w_chunks):
        col_sem += [sem_gs[k]] * sw
    sem_m = nc.alloc_semaphore("sem_m")
    sem_v = nc.alloc_semaphore("sem_v")
    sem_s = nc.alloc_semaphore("sem_s")

    # Input loads issue before the block-entry barrier: their DMA latency
    # overlaps the barrier instead of following it.
    if wc:
        nc.sync.dma_start(out=comb_sb, in_=comb_h.ap()[:, :]).then_inc(
            sem_c, 16
        )
    for h in range(2):
        nc.scalar.dma_start(out=eo_sb[h], in_=eo_hs[h].ap()[:, :]).then_inc(
            sem_es[h], 16
        )

    with nc.Block() as block:

        @block.sync
        def _(sync):
            # word stores
            for k, (s0, sw, rows) in enumerate(w_chunks):
                sync.wait_ge(sem_gs[k], 16 * sw)
                sync.dma_start(
                    out=out_ap[0:rows, s0 : s0 + sw, :],
                    in_=emb[0:rows, s0 : s0 + sw, :],
                ).then_inc(sem_s, 16)
            sync.wait_ge(sem_s, 16 * n_stores)

        @block.scalar
        def _(scalar):
            # entity stores
            b0 = 0
            for ew in e_chunks:
                scalar.wait_ge(sem_v, b0 + ew)
                scalar.dma_start(
                    out=out_ap[:, wc + b0 : wc + b0 + ew, :],
                    in_=emb[:, wc + b0 : wc + b0 + ew, :],
                ).then_inc(sem_s, 16)
                b0 += ew

        @block.gpsimd
        def _(gpsimd):
            if wc:
                gpsimd.wait_ge(sem_c, 16)
            for c in range(wc):
                rows = vp if c == wc - 1 else P
                gpsimd.indirect_dma_start(
                    out=emb[0:rows, c, :],
                    out_offset=None,
                    in_=tbl_ap[:, :],
                    in_offset=bass.IndirectOffsetOnAxis(
                        ap=comb_sb[0:rows, c : c + 1], axis=0
                    ),
                ).then_inc(col_sem[c], 16)

        @block.tensor
        def _(tensor):
            for b in range(B_LOC):
                h, j = divmod(b, half)
                tensor.wait_ge(sem_es[h], 16)
                tensor.matmul(
                    psum[b],
                    eo_sb[h][:, j * P : (j + 1) * P],
                    eo_sb[h][:, half * P + j * D : half * P + (j + 1) * D],
                    start=True,
                    stop=True,
                ).then_inc(sem_m, 1)

        @block.vector
        def _(vector):
            for b in range(B_LOC):
                vector.wait_ge(sem_m, b + 1)
                vector.tensor_copy(emb[:, wc + b, :], psum[b]).then_inc(
                    sem_v, 1
                )

    # Block exit emitted an all-engine barrier; now reset our semaphores so
    # the NEFF is re-executable.
    for s in (sem_c, *sem_es, *sem_gs, sem_m, sem_v, sem_s):
        nc.gpsimd.sem_clear(s)

    nc.compile()
    return nc


def _shard_inputs(caption_indices, entities_encoded, word_embedding,
                  pad_token, caption_masks):
    """Returns (wc, vp, in_maps, gather_toks_per_core, ent_toks_per_core)."""
    caption_indices = np.asarray(caption_indices, dtype=np.int32)
    caption_masks = np.asarray(caption_masks, dtype=np.int32)
    word_bf = np.asarray(word_embedding, dtype=np.float32).astype(BF16)
    ent_bf = np.asarray(entities_encoded, dtype=np.float32).astype(BF16)

    # Fused combined-table row index, computed exactly as the reference.
    idx = caption_indices                      # [B, L]
    msk = caption_masks[:, :, 0]               # [B, L]
    ent_i = np.where((idx - V < 0) | (idx - V >= N_ENT), N_ENT - 1, idx - V)
    word_i = np.where(idx >= V, np.int32(pad_token), idx)
    b_loc = (np.arange(B, dtype=np.int32) % B_LOC)[:, None]  # [B, 1]
    comb_full = np.where(
        msk == 1, V + N_ENT * b_loc + ent_i, word_i
    ).astype(np.int32)

    per_core = []
    wc_max, vp_max = 1, 1
    for i in range(N_CORES):
        sl = slice(i * B_LOC, (i + 1) * B_LOC)
        m = msk[sl].reshape(-1)                    # [1600]
        comb = comb_full[sl].reshape(-1)
        erow = ent_i[sl].reshape(-1)               # entity row within batch
        tok_b = np.arange(TOK) // L                # local batch id

        ent_toks = []      # per batch: array of token ids (<=128)
        spill = []
        for b in range(B_LOC):
            tb = np.nonzero((m == 1) & (tok_b == b))[0]
            ent_toks.append(tb[:P])
            spill.append(tb[P:])
        gather_toks = np.concatenate(
            [np.nonzero(m == 0)[0]] + spill
        )
        # sort by gathered row for HBM locality during the SDMA drain
        gather_toks = gather_toks[np.argsort(comb[gather_toks], kind="stable")]
        ng = len(gather_toks)
        wc = -(-ng // P) if ng else 0
        wc_max = max(wc_max, wc)
        per_core.append((sl, comb, erow, ent_toks, gather_toks))

    wc = wc_max  # one NEFF for all cores: use the max word-column count
    for (_, _, _, _, gt) in per_core:
        vp_max = max(vp_max, len(gt) - (wc - 1) * P)
    vp = vp_max

    in_maps = []
    gt_list, et_list = [], []
    half = B_LOC // 2
    for (sl, comb, erow, ent_toks, gather_toks) in per_core:
        tbl = np.concatenate(
            [word_bf, ent_bf[sl].reshape(B_LOC * N_ENT, D)], axis=0
        )
        cw = np.zeros(P * wc, dtype=np.int32)      # filler -> row 0
        cw[: len(gather_toks)] = comb[gather_toks]
        comb_w = np.ascontiguousarray(cw.reshape(wc, P).T)

        oh = np.zeros((N_ENT, B_LOC * P), dtype=BF16)
        for b in range(B_LOC):
            tb = ent_toks[b]
            oh[erow[tb], b * P + np.arange(len(tb))] = 1
        ent = ent_bf[sl].transpose(1, 0, 2).reshape(N_ENT, B_LOC * D)

        im = {"table": np.ascontiguousarray(tbl), "comb": comb_w}
        for h in range(2):
            im[f"entoh{h}"] = np.ascontiguousarray(
                np.concatenate(
                    [
                        oh[:, h * half * P : (h + 1) * half * P],
                        ent[:, h * half * D : (h + 1) * half * D],
                    ],
                    axis=1,
                )
            )
        in_maps.append(im)
        gt_list.append(gather_toks)
        et_list.append(ent_toks)
    return wc, vp, in_maps, gt_list, et_list


def _decode(res, wc, gather_toks, ent_toks):
    """res [P, wc+8, D] bf16 -> [TOK, D] f32 in original token order."""
    out = np.empty((TOK, D), dtype=np.float32)
    ng = len(gather_toks)
    if ng:
        g = (
            np.transpose(res[:, :wc, :], (1, 0, 2))
            .reshape(wc * P, D)[:ng]
            .astype(np.float32)
        )
        out[gather_toks] = g
    for b in range(B_LOC):
        tb = ent_toks[b]
        out[tb] = res[: len(tb), wc + b, :].astype(np.float32)
    return out


LAST_RESULTS = None  # BassKernelResults of the most recent run (for test.py)


def kernel(caption_indices, entities_encoded, word_embedding, pad_token,
           caption_masks):
    global LAST_RESULTS
    from concourse.bass_utils import run_bass_kernel_spmd

    wc, vp, in_maps, gt_list, et_list = _shard_inputs(
        caption_indices, entities_encoded, word_embedding, int(pad_token),
        caption_masks
    )
    nc = _build(wc, vp)
    res = run_bass_kernel_spmd(
        nc,
        in_maps,
        list(range(N_CORES)),
        trace=bool(os.environ.get("CAPEMB_TRACE")),
    )
    LAST_RESULTS = res
    out = np.empty((B, L, D), dtype=np.float32)
    for i in range(N_CORES):
        toks = _decode(res.results[i]["out"], wc, gt_list[i], et_list[i])
        out[i * B_LOC : (i + 1) * B_LOC] = toks.reshape(B_LOC, L, D)
    return out


# revision 15
# speedup vs baseline: 1.1351x; 1.0052x over previous
"""CaptionEmbedder kernel for Trainium2 (Bass), 8-core data-parallel.

Semantics (matching the reference):
    ent_idx  = clamp-to-49 of (caption_indices - 32000)   (oob -> 49)
    word_idx = caption_indices if < 32000 else pad_token
    out[b,l] = entities_encoded[b, ent_idx]  if caption_masks[b,l,0] == 1
               else word_embedding[word_idx]

Strategy: shard the batch dim (8 batches/core). Tokens are split between
two device-side mechanisms:

  * word tokens (mask==0) -- host packs them densely (sorted by row for
    HBM locality) into WC columns of 128 and the device runs WC native
    per-column indirect gathers (SWDGE; the Q7 descriptor-generation
    rate of ~1.4us/column dominates, so fewer columns = faster). The
    final column is partial (vp rows) so its drain - the critical tail -
    is short.
  * entity tokens (mask==1) -- grouped per local batch (<=128 each, one
    SBUF column per batch). The PE computes them as 8 tiny matmuls
    onehot[50,128].T @ entities_b[50,512] -> PSUM, evacuated to SBUF by
    the vector engine. Batches with >128 entity tokens spill the excess
    back into the gather path (combined table holds entity rows too).

All index math (fused combined-table row, token permutation, onehots)
is host-side numpy; the host inverts the permutation on the way out.
Everything travels as bfloat16 (halves HBM traffic, rel err ~4e-3);
the host up-casts the result to float32. Input loads issue before the
block-entry barrier so their latency hides under it; the onehot+entity
payload is split in two DMAs so the PE can start on the first half.
"""

import os
import sys
from functools import lru_cache

import numpy as np
import ml_dtypes

for _p in ("/opt/trn_rl_repo",):
    if _p not in sys.path:
        sys.path.insert(0, _p)

# Problem shapes (hardcoded per contest contract).
V = 32000          # vocab size
B = 64             # batch
L = 200            # caption length
N_ENT = 50         # entities per batch
D = 512            # embedding dim
N_CORES = 8
B_LOC = B // N_CORES            # 8 batches per core
TOK = B_LOC * L                 # 1600 tokens per core
P = 128                         # SBUF partitions
TBL = V + B_LOC * N_ENT         # 32400 rows in combined table
EOH = B_LOC * (P + D) // 2      # onehot+entities free dim per half (2560)

BF16 = ml_dtypes.bfloat16


@lru_cache(maxsize=4)
def _build(wc: int, vp: int):
    """wc word-gather columns; the last one holds only vp<=128 rows."""
    import concourse.bacc as bacc
    import concourse.bass as bass
    from concourse import mybir

    i32 = mybir.dt.int32
    bf16 = mybir.dt.bfloat16
    f32 = mybir.dt.float32

    cols = wc + B_LOC  # word columns + one entity column per local batch
    half = B_LOC // 2

    nc = bacc.Bacc("TRN2", target_bir_lowering=False, debug=False)

    tbl_h = nc.dram_tensor("table", [TBL, D], bf16, kind="ExternalInput")
    comb_h = nc.dram_tensor("comb", [P, max(wc, 1)], i32, kind="ExternalInput")
    # onehot+entities, one tensor per half: [50, 4*128 | 4*512]
    eo_hs = [
        nc.dram_tensor(f"entoh{h}", [N_ENT, EOH], bf16, kind="ExternalInput")
        for h in range(2)
    ]
    out_h = nc.dram_tensor("out", [P, cols, D], bf16, kind="ExternalOutput")
    tbl_ap = tbl_h.ap()
    out_ap = out_h.ap()

    comb_sb = nc.alloc_sbuf_tensor("comb_sb", [P, max(wc, 1)], i32).ap()
    eo_sb = [
        nc.alloc_sbuf_tensor(f"eo_sb{h}", [N_ENT, EOH], bf16).ap()
        for h in range(2)
    ]
    emb = nc.alloc_sbuf_tensor("emb", [P, cols, D], bf16).ap()
    psum = [
        nc.alloc_psum_tensor(f"ps{b}", [P, D], f32).ap() for b in range(B_LOC)
    ]

    # word store chunks: full columns in chunks of <=2 (early, small store
    # traffic keeps SDMA clear for the gather-tail sem), partial tail alone
    full = wc - 1
    w_chunks = []
    s = 0
    while s < full:
        c = min(2, full - s)
        w_chunks.append((s, c, P))
        s += c
    w_chunks.append((full, 1, vp))
    e_chunks = (4, 4)
    n_stores = len(w_chunks) + len(e_chunks)

    sem_c = nc.alloc_semaphore("sem_c")
    sem_es = [nc.alloc_semaphore(f"sem_e{h}") for h in range(2)]
    # one gather sem per store chunk (sum of 16-incs is order-independent)
    sem_gs = [nc.alloc_semaphore(f"sem_g{k}") for k in range(len(w_chunks))]
    col_sem = []  # column index -> chunk sem
    for k, (s0, sw, rows) in enumerate(w_chunks):
        col_sem += [sem_gs[k]] * sw
    sem_m = nc.alloc_semaphore("sem_m")
    sem_v = nc.alloc_semaphore("sem_v")
    sem_s = nc.alloc_semaphore("sem_s")

    # Input loads issue before the block-entry barrier: their DMA latency
    # overlaps the barrier instead of following it.
    if wc:
        nc.sync.dma_start(out=comb_sb, in_=comb_h.ap()[:, :]).then_inc(
            sem_c, 16
        )
    for h in range(2):
        nc.scalar.dma_start(out=eo_sb[h], in_=eo_hs[h].ap()[:, :]).then_inc(
            sem_es[h], 16
        )

    with nc.Block(no_gpsimd_drain=True) as block:

        sync_ks = [0, len(w_chunks) - 1] if len(w_chunks) > 1 else [0]
        vec_ks = [k for k in range(len(w_chunks)) if k not in sync_ks]

        def _w_store(eng, k):
            s0, sw, rows = w_chunks[k]
            eng.wait_ge(sem_gs[k], 16 * sw)
            eng.dma_start(
                out=out_ap[0:rows, s0 : s0 + sw, :],
                in_=emb[0:rows, s0 : s0 + sw, :],
            ).then_inc(sem_s, 16)

        @block.sync
        def _(sync):
            # first word chunk + the partial tail; final completion wait
            for k in sync_ks:
                _w_store(sync, k)
            sync.wait_ge(sem_s, 16 * n_stores)

        @block.scalar
        def _(scalar):
            # entity stores, then the middle word chunks
            b0 = 0
            for ew in e_chunks:
                scalar.wait_ge(sem_v, b0 + ew)
                scalar.dma_start(
                    out=out_ap[:, wc + b0 : wc + b0 + ew, :],
                    in_=emb[:, wc + b0 : wc + b0 + ew, :],
                ).then_inc(sem_s, 16)
                b0 += ew
            for k in vec_ks:
                _w_store(scalar, k)

        @block.gpsimd
        def _(gpsimd):
            if wc:
                gpsimd.wait_ge(sem_c, 16)
            for c in range(wc):
                rows = vp if c == wc - 1 else P
                gpsimd.indirect_dma_start(
                    out=emb[0:rows, c, :],
                    out_offset=None,
                    in_=tbl_ap[:, :],
                    in_offset=bass.IndirectOffsetOnAxis(
                        ap=comb_sb[0:rows, c : c + 1], axis=0
                    ),
                ).then_inc(col_sem[c], 16)

        @block.tensor
        def _(tensor):
            for b in range(B_LOC):
                h, j = divmod(b, half)
                tensor.wait_ge(sem_es[h], 16)
                tensor.matmul(
                    psum[b],
                    eo_sb[h][:, j * P : (j + 1) * P],
                    eo_sb[h][:, half * P + j * D : half * P + (j + 1) * D],
                    start=True,
                    stop=True,
                ).then_inc(sem_m, 1)

        @block.vector
        def _(vector):
            for b in range(B_LOC):
                vector.wait_ge(sem_m, b + 1)
                vector.tensor_copy(emb[:, wc + b, :], psum[b]).then_inc(
                    sem_v, 1
                )

    # Block exit emitted an all-engine barrier; now reset our semaphores so
    # the NEFF is re-executable (one range-clear: ids are contiguous).
    all_sems = [sem_c, *sem_es, *sem_gs, sem_m, sem_v, sem_s]
    nums = sorted(s.num for s in all_sems)
    assert nums == list(range(nums[0], nums[0] + len(nums)))
    nc.gpsimd.sem_clear(range(nums[0], nums[-1] + 1))

    nc.compile()
    return nc


def _shard_inputs(caption_indices, entities_encoded, word_embedding,
                  pad_token, caption_masks):
    """Returns (wc, vp, in_maps, gather_toks_per_core, ent_toks_per_core)."""
    caption_indices = np.asarray(caption_indices, dtype=np.int32)
    caption_masks = np.asarray(caption_masks, dtype=np.int32)
    word_bf = np.asarray(word_embedding, dtype=np.float32).astype(BF16)
    ent_bf = np.asarray(entities_encoded, dtype=np.float32).astype(BF16)

    # Fused combined-table row index, computed exactly as the reference.
    idx = caption_indices                      # [B, L]
    msk = caption_masks[:, :, 0]               # [B, L]
    ent_i = np.where((idx - V < 0) | (idx - V >= N_ENT), N_ENT - 1, idx - V)
    word_i = np.where(idx >= V, np.int32(pad_token), idx)
    b_loc = (np.arange(B, dtype=np.int32) % B_LOC)[:, None]  # [B, 1]
    comb_full = np.where(
        msk == 1, V + N_ENT * b_loc + ent_i, word_i
    ).astype(np.int32)

    per_core = []
    wc_max, vp_max = 1, 1
    for i in range(N_CORES):
        sl = slice(i * B_LOC, (i + 1) * B_LOC)
        m = msk[sl].reshape(-1)                    # [1600]
        comb = comb_full[sl].reshape(-1)
        erow = ent_i[sl].reshape(-1)               # entity row within batch
        tok_b = np.arange(TOK) // L                # local batch id

        ent_toks = []      # per batch: array of token ids (<=128)
        spill = []
        for b in range(B_LOC):
            tb = np.nonzero((m == 1) & (tok_b == b))[0]
            ent_toks.append(tb[:P])
            spill.append(tb[P:])
        gather_toks = np.concatenate(
            [np.nonzero(m == 0)[0]] + spill
        )
        # sort by gathered row for HBM locality during the SDMA drain
        gather_toks = gather_toks[np.argsort(comb[gather_toks], kind="stable")]
        ng = len(gather_toks)
        wc = -(-ng // P) if ng else 0
        wc_max = max(wc_max, wc)
        per_core.append((sl, comb, erow, ent_toks, gather_toks))

    wc = wc_max  # one NEFF for all cores: use the max word-column count
    for (_, _, _, _, gt) in per_core:
        vp_max = max(vp_max, len(gt) - (wc - 1) * P)
    vp = vp_max

    in_maps = []
    gt_list, et_list = [], []
    half = B_LOC // 2
    for (sl, comb, erow, ent_toks, gather_toks) in per_core:
        tbl = np.concatenate(
            [word_bf, ent_bf[sl].reshape(B_LOC * N_ENT, D)], axis=0
        )
        cw = np.zeros(P * wc, dtype=np.int32)      # filler -> row 0
        cw[: len(gather_toks)] = comb[gather_toks]
        comb_w = np.ascontiguousarray(cw.reshape(wc, P).T)

        oh = np.zeros((N_ENT, B_LOC * P), dtype=BF16)
        for b in range(B_LOC):
            tb = ent_toks[b]
            oh[erow[tb], b * P + np.arange(len(tb))] = 1
        ent = ent_bf[sl].transpose(1, 0, 2).reshape(N_ENT, B_LOC * D)

        im = {"table": np.ascontiguousarray(tbl), "comb": comb_w}
        for h in range(2):
            im[f"entoh{h}"] = np.ascontiguousarray(
                np.concatenate(
                    [
                        oh[:, h * half * P : (h + 1) * half * P],
                        ent[:, h * half * D : (h + 1) * half * D],
                    ],
                    axis=1,
                )
            )
        in_maps.append(im)
        gt_list.append(gather_toks)
        et_list.append(ent_toks)
    return wc, vp, in_maps, gt_list, et_list


def _decode(res, wc, gather_toks, ent_toks):
    """res [P, wc+8, D] bf16 -> [TOK, D] f32 in original token order."""
    out = np.empty((TOK, D), dtype=np.float32)
    ng = len(gather_toks)
    if ng:
        g = (
            np.transpose(res[:, :wc, :], (1, 0, 2))
            .reshape(wc * P, D)[:ng]
            .astype(np.float32)
        )
        out[gather_toks] = g
    for b in range(B_LOC):
        tb = ent_toks[b]
        out[tb] = res[: len(tb), wc + b, :].astype(np.float32)
    return out


LAST_RESULTS = None  # BassKernelResults of the most recent run (for test.py)


def kernel(caption_indices, entities_encoded, word_embedding, pad_token,
           caption_masks):
    global LAST_RESULTS
    from concourse.bass_utils import run_bass_kernel_spmd

    wc, vp, in_maps, gt_list, et_list = _shard_inputs(
        caption_indices, entities_encoded, word_embedding, int(pad_token),
        caption_masks
    )
    nc = _build(wc, vp)
    res = run_bass_kernel_spmd(
        nc,
        in_maps,
        list(range(N_CORES)),
        trace=bool(os.environ.get("CAPEMB_TRACE")),
    )
    LAST_RESULTS = res
    out = np.empty((B, L, D), dtype=np.float32)
    for i in range(N_CORES):
        toks = _decode(res.results[i]["out"], wc, gt_list[i], et_list[i])
        out[i * B_LOC : (i + 1) * B_LOC] = toks.reshape(B_LOC, L, D)
    return out


# revision 16
# speedup vs baseline: 1.1605x; 1.0224x over previous
"""CaptionEmbedder kernel for Trainium2 (Bass), 8-core data-parallel.

Semantics (matching the reference):
    ent_idx  = clamp-to-49 of (caption_indices - 32000)   (oob -> 49)
    word_idx = caption_indices if < 32000 else pad_token
    out[b,l] = entities_encoded[b, ent_idx]  if caption_masks[b,l,0] == 1
               else word_embedding[word_idx]

Strategy: shard the batch dim (8 batches/core). Tokens are split between
two device-side mechanisms:

  * word tokens (mask==0) -- host packs them densely (sorted by row for
    HBM locality) into WC columns of 128 and the device runs WC native
    per-column indirect gathers (SWDGE; the Q7 descriptor-generation
    rate of ~1.4us/column dominates, so fewer columns = faster). The
    final column is partial (vp rows) so its drain - the critical tail -
    is short.
  * entity tokens (mask==1) -- grouped per local batch (<=128 each, one
    SBUF column per batch). The PE computes them as 8 tiny matmuls
    onehot[50,128].T @ entities_b[50,512] -> PSUM, evacuated to SBUF by
    the vector engine. Batches with >128 entity tokens spill the excess
    back into the gather path (combined table holds entity rows too).

All index math (fused combined-table row, token permutation, onehots)
is host-side numpy; the host inverts the permutation on the way out.
Everything travels as bfloat16 (halves HBM traffic, rel err ~4e-3);
the host up-casts the result to float32. Input loads issue before the
block-entry barrier so their latency hides under it; the onehot+entity
payload is split in two DMAs so the PE can start on the first half.
"""

import os
import sys
from functools import lru_cache

import numpy as np
import ml_dtypes

for _p in ("/opt/trn_rl_repo",):
    if _p not in sys.path:
        sys.path.insert(0, _p)

# Problem shapes (hardcoded per contest contract).
V = 32000          # vocab size
B = 64             # batch
L = 200            # caption length
N_ENT = 50         # entities per batch
D = 512            # embedding dim
N_CORES = 8
B_LOC = B // N_CORES            # 8 batches per core
TOK = B_LOC * L                 # 1600 tokens per core
P = 128                         # SBUF partitions
TBL = V + B_LOC * N_ENT         # 32400 rows in combined table
EOH = B_LOC * (P + D) // 2      # onehot+entities free dim per half (2560)

BF16 = ml_dtypes.bfloat16


@lru_cache(maxsize=4)
def _build(wc: int, vp: int):
    """wc word-gather columns; the last one holds only vp<=128 rows."""
    import concourse.bacc as bacc
    import concourse.bass as bass
    from concourse import mybir

    i32 = mybir.dt.int32
    bf16 = mybir.dt.bfloat16
    f32 = mybir.dt.float32

    cols = wc + B_LOC  # word columns + one entity column per local batch
    half = B_LOC // 2

    nc = bacc.Bacc("TRN2", target_bir_lowering=False, debug=False)

    tbl_h = nc.dram_tensor("table", [TBL, D], bf16, kind="ExternalInput")
    comb_h = nc.dram_tensor("comb", [P, max(wc, 1)], i32, kind="ExternalInput")
    # onehot+entities, one tensor per half: [50, 4*128 | 4*512]
    eo_hs = [
        nc.dram_tensor(f"entoh{h}", [N_ENT, EOH], bf16, kind="ExternalInput")
        for h in range(2)
    ]
    out_h = nc.dram_tensor("out", [P, cols, D], bf16, kind="ExternalOutput")
    tbl_ap = tbl_h.ap()
    out_ap = out_h.ap()

    comb_sb = nc.alloc_sbuf_tensor("comb_sb", [P, max(wc, 1)], i32).ap()
    eo_sb = [
        nc.alloc_sbuf_tensor(f"eo_sb{h}", [N_ENT, EOH], bf16).ap()
        for h in range(2)
    ]
    emb = nc.alloc_sbuf_tensor("emb", [P, cols, D], bf16).ap()
    psum = [
        nc.alloc_psum_tensor(f"ps{b}", [P, D], f32).ap() for b in range(B_LOC)
    ]

    # word store chunks: full columns in chunks of <=2 (early, small store
    # traffic keeps SDMA clear for the gather-tail sem), partial tail alone
    full = wc - 1
    w_chunks = []
    s = 0
    while s < full:
        c = min(2, full - s)
        w_chunks.append((s, c, P))
        s += c
    w_chunks.append((full, 1, vp))
    e_chunks = (4, 4)
    n_stores = len(w_chunks) + len(e_chunks)

    sem_c = nc.alloc_semaphore("sem_c")
    sem_es = [nc.alloc_semaphore(f"sem_e{h}") for h in range(2)]
    # one gather sem per store chunk (sum of 16-incs is order-independent)
    sem_gs = [nc.alloc_semaphore(f"sem_g{k}") for k in range(len(w_chunks))]
    col_sem = []  # column index -> chunk sem
    for k, (s0, sw, rows) in enumerate(w_chunks):
        col_sem += [sem_gs[k]] * sw
    sem_m = nc.alloc_semaphore("sem_m")
    sem_v = nc.alloc_semaphore("sem_v")
    sem_s = nc.alloc_semaphore("sem_s")

    # Input loads issue before the block-entry barrier: their DMA latency
    # overlaps the barrier instead of following it.
    if wc:
        nc.sync.dma_start(out=comb_sb, in_=comb_h.ap()[:, :]).then_inc(
            sem_c, 16
        )
    for h in range(2):
        nc.scalar.dma_start(out=eo_sb[h], in_=eo_hs[h].ap()[:, :]).then_inc(
            sem_es[h], 16
        )

    with nc.Block(no_gpsimd_drain=True) as block:

        sync_ks = [0, len(w_chunks) - 1] if len(w_chunks) > 1 else [0]
        vec_ks = [k for k in range(len(w_chunks)) if k not in sync_ks]

        def _w_store(eng, k):
            s0, sw, rows = w_chunks[k]
            eng.wait_ge(sem_gs[k], 16 * sw)
            eng.dma_start(
                out=out_ap[0:rows, s0 : s0 + sw, :],
                in_=emb[0:rows, s0 : s0 + sw, :],
            ).then_inc(sem_s, 16)

        @block.sync
        def _(sync):
            # first word chunk + the partial tail; final completion wait
            for k in sync_ks:
                _w_store(sync, k)
            sync.wait_ge(sem_s, 16 * n_stores)

        @block.scalar
        def _(scalar):
            # entity stores, then the middle word chunks
            b0 = 0
            for ew in e_chunks:
                scalar.wait_ge(sem_v, b0 + ew)
                scalar.dma_start(
                    out=out_ap[:, wc + b0 : wc + b0 + ew, :],
                    in_=emb[:, wc + b0 : wc + b0 + ew, :],
                ).then_inc(sem_s, 16)
                b0 += ew
            for k in vec_ks:
                _w_store(scalar, k)

        @block.gpsimd
        def _(gpsimd):
            if wc:
                gpsimd.wait_ge(sem_c, 16)
            for c in range(wc):
                rows = vp if c == wc - 1 else P
                gpsimd.indirect_dma_start(
                    out=emb[0:rows, c, :],
                    out_offset=None,
                    in_=tbl_ap[:, :],
                    in_offset=bass.IndirectOffsetOnAxis(
                        ap=comb_sb[0:rows, c : c + 1], axis=0
                    ),
                ).then_inc(col_sem[c], 16)

        @block.tensor
        def _(tensor):
            for b in range(B_LOC):
                h, j = divmod(b, half)
                tensor.wait_ge(sem_es[h], 16)
                tensor.matmul(
                    psum[b],
                    eo_sb[h][:, j * P : (j + 1) * P],
                    eo_sb[h][:, half * P + j * D : half * P + (j + 1) * D],
                    start=True,
                    stop=True,
                ).then_inc(sem_m, 1)

        @block.vector
        def _(vector):
            for b in range(B_LOC):
                vector.wait_ge(sem_m, b + 1)
                vector.tensor_copy(emb[:, wc + b, :], psum[b]).then_inc(
                    sem_v, 1
                )

    # Block exit emitted an all-engine barrier; now reset our semaphores so
    # the NEFF is re-executable (one range-clear: ids are contiguous).
    all_sems = [sem_c, *sem_es, *sem_gs, sem_m, sem_v, sem_s]
    nums = sorted(s.num for s in all_sems)
    assert nums == list(range(nums[0], nums[0] + len(nums)))
    nc.gpsimd.sem_clear(range(nums[0], nums[-1] + 1))

    nc.compile()
    return nc


def _shard_inputs(caption_indices, entities_encoded, word_embedding,
                  pad_token, caption_masks):
    """Returns (wc, vp, in_maps, gt_list, et_list).

    Word tokens are core-agnostic (the word table is replicated), so they
    are pooled globally, sorted by row for HBM locality, and dealt
    round-robin so every core gathers the same count (exec time is the
    max over cores). Entity tokens and any per-batch spill stay on their
    home core (their rows live in that core's table section).
    gt_list[i] holds GLOBAL token ids (0..B*L)."""
    caption_indices = np.asarray(caption_indices, dtype=np.int32)
    caption_masks = np.asarray(caption_masks, dtype=np.int32)
    word_bf = np.asarray(word_embedding, dtype=np.float32).astype(BF16)
    ent_bf = np.asarray(entities_encoded, dtype=np.float32).astype(BF16)

    # Fused combined-table row index, computed exactly as the reference.
    idx = caption_indices                      # [B, L]
    msk = caption_masks[:, :, 0]               # [B, L]
    ent_i = np.where((idx - V < 0) | (idx - V >= N_ENT), N_ENT - 1, idx - V)
    word_i = np.where(idx >= V, np.int32(pad_token), idx)

    # global word-token pool, sorted by row, dealt round-robin
    msk_flat = msk.reshape(-1)
    wrows_flat = word_i.reshape(-1)
    word_g = np.nonzero(msk_flat == 0)[0].astype(np.int64)
    word_g = word_g[np.argsort(wrows_flat[word_g], kind="stable")]
    assign = [word_g[i::N_CORES] for i in range(N_CORES)]

    per_core = []
    for i in range(N_CORES):
        sl = slice(i * B_LOC, (i + 1) * B_LOC)
        m = msk[sl].reshape(-1)                    # [1600] local
        erow = ent_i[sl].reshape(-1)               # entity row within batch
        tok_b = np.arange(TOK) // L                # local batch id

        ent_toks = []      # per batch: local token ids (<=128)
        spill = []
        for b in range(B_LOC):
            tb = np.nonzero((m == 1) & (tok_b == b))[0]
            ent_toks.append(tb[:P])
            spill.append(tb[P:])
        spill = np.concatenate(spill) if spill else np.empty(0, np.int64)
        # gather list: assigned global word tokens (rows ascending), then
        # own spill (entity rows >= V, so overall order stays ascending)
        spill_rows = (V + N_ENT * (spill // L) + erow[spill]).astype(np.int32)
        gt_global = np.concatenate([assign[i], i * TOK + spill])
        rows = np.concatenate([wrows_flat[assign[i]], spill_rows])
        per_core.append((sl, rows, erow, ent_toks, gt_global))

    wc = max(1, max(-(-len(r) // P) for (_, r, _, _, _) in per_core))
    vp = max(1, max(len(r) - (wc - 1) * P for (_, r, _, _, _) in per_core))

    in_maps = []
    gt_list, et_list = [], []
    half = B_LOC // 2
    for (sl, rows, erow, ent_toks, gt_global) in per_core:
        tbl = np.concatenate(
            [word_bf, ent_bf[sl].reshape(B_LOC * N_ENT, D)], axis=0
        )
        cw = np.zeros(P * wc, dtype=np.int32)      # filler -> row 0
        cw[: len(rows)] = rows
        comb_w = np.ascontiguousarray(cw.reshape(wc, P).T)

        oh = np.zeros((N_ENT, B_LOC * P), dtype=BF16)
        for b in range(B_LOC):
            tb = ent_toks[b]
            oh[erow[tb], b * P + np.arange(len(tb))] = 1
        ent = ent_bf[sl].transpose(1, 0, 2).reshape(N_ENT, B_LOC * D)

        im = {"table": np.ascontiguousarray(tbl), "comb": comb_w}
        for h in range(2):
            im[f"entoh{h}"] = np.ascontiguousarray(
                np.concatenate(
                    [
                        oh[:, h * half * P : (h + 1) * half * P],
                        ent[:, h * half * D : (h + 1) * half * D],
                    ],
                    axis=1,
                )
            )
        in_maps.append(im)
        gt_list.append(gt_global)
        et_list.append(ent_toks)
    return wc, vp, in_maps, gt_list, et_list


def _decode_into(out_flat, res, wc, gather_g, ent_toks, core):
    """Scatter one core's result [P, wc+8, D] into out_flat [B*L, D]."""
    ng = len(gather_g)
    if ng:
        g = (
            np.transpose(res[:, :wc, :], (1, 0, 2))
            .reshape(wc * P, D)[:ng]
            .astype(np.float32)
        )
        out_flat[gather_g] = g
    for b in range(B_LOC):
        tb = ent_toks[b]
        out_flat[core * TOK + tb] = res[: len(tb), wc + b, :].astype(
            np.float32
        )


LAST_RESULTS = None  # BassKernelResults of the most recent run (for test.py)


def kernel(caption_indices, entities_encoded, word_embedding, pad_token,
           caption_masks):
    global LAST_RESULTS
    from concourse.bass_utils import run_bass_kernel_spmd

    wc, vp, in_maps, gt_list, et_list = _shard_inputs(
        caption_indices, entities_encoded, word_embedding, int(pad_token),
        caption_masks
    )
    nc = _build(wc, vp)
    res = run_bass_kernel_spmd(
        nc,
        in_maps,
        list(range(N_CORES)),
        trace=bool(os.environ.get("CAPEMB_TRACE")),
    )
    LAST_RESULTS = res
    out_flat = np.empty((B * L, D), dtype=np.float32)
    for i in range(N_CORES):
        _decode_into(out_flat, res.results[i]["out"], wc, gt_list[i],
                     et_list[i], i)
    return out_flat.reshape(B, L, D)
